# revision 5
# baseline (speedup 1.0000x reference)
"""Trainium2 Bass kernel for int8-STE fake-quant Conv2d (LUT forward path).

Math: the reference's LUT is the exact int8 product table
lut[i, j] = (i-128)*(j-128), indexed by (q+128), so its LUT path equals
    out = (sf*sw) * conv2d(qx, qw, pad=1) + bias,
with sf = max|x|/127, sw = max|w|/127, qx = round(x/sf), qw = round(w/sw)
(the clip never binds because |x/sf| <= 127).  qx, qw are small integers,
exact in bf16; PSUM accumulates in fp32, so the integer conv is computed
exactly on the PE array.

Sharding: data-parallel over batch N (8 batches -> 8 cores).  The dynamic
per-tensor scale needs a global absmax over x; a cross-core collective for
4 bytes has a ~20us latency floor on trn2, so instead every core also
reads the other 7 batches once and reduces the global max locally
(~1MB extra DMA ~= 3us, far cheaper).

Per-core layout (host-side prep, layout-only transforms):
  xr [32,1232] fp32: own batch zero-padded to 34x34, flattened [32,1156],
       zero tail.
  xo [224,1024] fp32: the other 7 batches, raw (absmax input only).
  wc [32,288]   fp32: weight as [cin, (ky kx o)].
  bs [128,1]    fp32: bias per output channel, replicated x4 (psum rows).
Conv: 9 taps as shifted windows of the padded flat image; 4 row-chunks of
the image run concurrently in 4 PE column-groups (tile_position), taps
accumulate serially into one PSUM bank [128, 512] (chunk c at partitions
32c.. covers output rows 8c..8c+7, free dim 272 = 8 rows of 34).
"""

import numpy as np

_CACHE = {}

QMAGIC = float(1.5 * 2**23)  # fp32 round-to-nearest-even magic constant


def _build_module():
    from contextlib import ExitStack

    import concourse.bacc as bacc
    import concourse.mybir as mybir
    import concourse.tile as tile
    from concourse import bass_isa

    f32 = mybir.dt.float32
    bf16 = mybir.dt.bfloat16
    AX = mybir.AxisListType
    OP = mybir.AluOpType
    ACT = mybir.ActivationFunctionType

    nc = bacc.Bacc("TRN2", target_bir_lowering=False, debug=False)
    xo = nc.dram_tensor("xo", [224, 1024], f32, kind="ExternalInput")
    xr = nc.dram_tensor("xr", [32, 1232], f32, kind="ExternalInput")
    wc = nc.dram_tensor("wc", [32, 288], f32, kind="ExternalInput")
    bs = nc.dram_tensor("bs", [128, 1], f32, kind="ExternalInput")
    y = nc.dram_tensor("y", [32, 1024], f32, kind="ExternalOutput")

    with tile.TileContext(nc) as tc:
        with ExitStack() as ctx:
            sb = ctx.enter_context(tc.tile_pool(name="sb", bufs=1))
            ps = ctx.enter_context(tc.tile_pool(name="ps", bufs=1, space="PSUM"))

            O1 = sb.tile([128, 1024], f32, tag="o1")
            O2 = sb.tile([96, 1024], f32, tag="o2")
            R = sb.tile([32, 1232], f32, tag="r")
            W4 = sb.tile([32, 288], f32, tag="w4")
            BS = sb.tile([128, 1], f32, tag="bs")
            nc.sync.dma_start(O1[:, :], xo[0:128, :])
            nc.sync.dma_start(O2[:, :], xo[128:224, :])
            nc.sync.dma_start(R[:, :], xr[:, :])
            nc.sync.dma_start(W4[:, :], wc[:, :])
            nc.sync.dma_start(BS[:, :], bs[:, :])

            # ---- global absmax of x (cols 0:3) and w (col 3) ----
            S = sb.tile([128, 4], f32, tag="s")
            nc.vector.memset(S[:, :], 0.0)
            nc.vector.tensor_reduce(
                S[:, 0:1], O1[:, :], axis=AX.X, op=OP.max, apply_absolute_value=True
            )
            nc.vector.tensor_reduce(
                S[0:96, 1:2], O2[:, :], axis=AX.X, op=OP.max, apply_absolute_value=True
            )
            nc.vector.tensor_reduce(
                S[0:32, 2:3], R[:, :], axis=AX.X, op=OP.max, apply_absolute_value=True
            )
            nc.vector.tensor_reduce(
                S[0:32, 3:4], W4[:, :], axis=AX.X, op=OP.max, apply_absolute_value=True
            )
            U = sb.tile([128, 2], f32, tag="u")
            nc.vector.tensor_reduce(
                U[:, 0:1], S[:, 0:3], axis=AX.X, op=OP.max, apply_absolute_value=True
            )
            nc.vector.tensor_copy(U[:, 1:2], S[:, 3:4])
            BC = sb.tile([128, 2], f32, tag="bc")
            nc.gpsimd.partition_all_reduce(
                BC[:, :], U[:, :], channels=128, reduce_op=bass_isa.ReduceOp.max
            )

            # ---- scales: SQ = [127/m_x, 127/m_w], SO = m_x*m_w/127^2 ----
            RC = sb.tile([128, 2], f32, tag="rc")
            nc.vector.reciprocal(RC[:, :], BC[:, :])
            SQ = sb.tile([128, 2], f32, tag="sq")
            nc.vector.tensor_scalar_mul(SQ[:, :], RC[:, :], 127.0)
            T0 = sb.tile([128, 1], f32, tag="t0")
            nc.vector.tensor_mul(T0[:, :], BC[:, 0:1], BC[:, 1:2])
            SO = sb.tile([128, 1], f32, tag="so")
            nc.vector.tensor_scalar_mul(SO[:, :], T0[:, :], 1.0 / 16129.0)

            # ---- quantize: q = (v*scale + MAGIC) - MAGIC, cast bf16 ----
            TMP = sb.tile([32, 1232], f32, tag="tmp")
            nc.vector.tensor_scalar(
                TMP[:, :], R[:, :], SQ[0:32, 0:1], QMAGIC, op0=OP.mult, op1=OP.add
            )
            XQ = sb.tile([32, 1232], bf16, tag="xq")
            nc.scalar.activation(XQ[:, :], TMP[:, :], ACT.Copy, bias=-QMAGIC)
            TMPW = sb.tile([32, 288], f32, tag="tmpw")
            nc.vector.tensor_scalar(
                TMPW[:, :], W4[:, :], SQ[0:32, 1:2], QMAGIC, op0=OP.mult, op1=OP.add
            )
            WQ = sb.tile([32, 288], bf16, tag="wq")
            nc.scalar.activation(WQ[:, :], TMPW[:, :], ACT.Copy, bias=-QMAGIC)

            # ---- conv: 4 image chunks in 4 PE col-groups, 9 serial taps ----
            P1 = ps.tile([128, 512], f32, tag="p1")
            for t in range(9):
                off = (t // 3) * 34 + (t % 3)
                for c4 in range(4):
                    nc.tensor.matmul(
                        P1[32 * c4 : 32 * c4 + 32, 0:272],
                        WQ[:, t * 32 : (t + 1) * 32],
                        XQ[:, 272 * c4 + off : 272 * c4 + off + 272],
                        start=(t == 0),
                        stop=(t == 8),
                        tile_position=(0, 32 * c4),
                        skip_group_check=True,
                    )

            # ---- dequant + bias ----
            Y = sb.tile([128, 272], f32, tag="y")
            nc.vector.tensor_scalar(
                Y[:, :], P1[:, 0:272], SO[:, 0:1], BS[:, 0:1], op0=OP.mult, op1=OP.add
            )

            # ---- store valid region: chunk c -> output rows 8c..8c+7 ----
            for c4 in range(4):
                src = Y[32 * c4 : 32 * c4 + 32, :].rearrange(
                    "o (yy g) -> o yy g", yy=8, g=34
                )[:, :, 0:32]
                dst = y.ap().rearrange("o (cc yy w) -> o cc yy w", cc=4, yy=8, w=32)[
                    :, c4, :, :
                ]
                nc.sync.dma_start(dst, src)

    nc.compile()
    return nc


def _get_module():
    if "nc" not in _CACHE:
        _CACHE["nc"] = _build_module()
    return _CACHE["nc"]


def _prep_inputs(x, weight, bias):
    """Host-side sharding/layout prep (layout-only transforms)."""
    n_cores = 8
    x = np.asarray(x, dtype=np.float32)
    weight = np.asarray(weight, dtype=np.float32)
    bias = np.asarray(bias, dtype=np.float32)

    wc = np.ascontiguousarray(
        weight.transpose(1, 2, 3, 0).reshape(32, 288)
    )  # [cin, (ky kx o)]
    bsr = np.tile(bias.reshape(32, 1), (4, 1))  # [128, 1]

    xpad = np.pad(x, ((0, 0), (0, 0), (1, 1), (1, 1)))  # [8,32,34,34]
    xpad_flat = xpad.reshape(8, 32, 1156)
    x_flat = x.reshape(8, 32, 1024)

    in_maps = []
    for i in range(n_cores):
        xrow = np.zeros((32, 1232), np.float32)
        xrow[:, :1156] = xpad_flat[i]
        others = np.ascontiguousarray(x_flat[np.arange(8) != i].reshape(224, 1024))
        in_maps.append({"xo": others, "xr": xrow, "wc": wc, "bs": bsr})
    return in_maps


def _lut_is_product_table(lut):
    lut = np.asarray(lut)
    i = np.arange(256, dtype=np.float32) - 128.0
    return lut.shape == (256, 256) and np.array_equal(
        lut.astype(np.float32), i[:, None] * i[None, :]
    )


def _host_fallback(x, weight, bias, lut):
    """Exact numpy emulation of the reference for a non-product-table lut.
    Only used if the lut assumption is violated (never for the shipped
    setup_inputs); keeps kernel() correct under any lut."""
    x = np.asarray(x, np.float32)
    weight = np.asarray(weight, np.float32)
    bias = np.asarray(bias, np.float32)
    lut = np.asarray(lut, np.float32)
    sf = np.abs(x).max() / np.float32(127.0)
    sw = np.abs(weight).max() / np.float32(127.0)
    qx = np.clip(np.round(x / sf), -128, 127)
    qw = np.clip(np.round(weight / sw), -128, 127)
    N, C, H, W = x.shape
    O = weight.shape[0]
    xp = np.pad(qx, ((0, 0), (0, 0), (1, 1), (1, 1)))
    ix = np.zeros((N, C, 3, 3, H, W), np.int32)
    for ky in range(3):
        for kx in range(3):
            ix[:, :, ky, kx] = xp[:, :, ky : ky + H, kx : kx + W].astype(np.int32) + 128
    iw = (qw.astype(np.int32) + 128).reshape(O, C, 3, 3)
    out = np.zeros((N, O, H, W), np.float32)
    for o in range(O):
        out[:, o] = lut[ix, iw[o][None, :, :, :, None, None]].sum(axis=(1, 2, 3))
    out = (sf * sw) * out + bias[None, :, None, None]
    return out.astype(np.float32)


def kernel(x, weight, bias, lut):
    x = np.asarray(x, np.float32)
    weight = np.asarray(weight, np.float32)
    bias = np.asarray(bias, np.float32)

    if not _lut_is_product_table(lut):
        return _host_fallback(x, weight, bias, lut)

    from concourse.bass_utils import run_bass_kernel_spmd

    nc = _get_module()
    in_maps = _prep_inputs(x, weight, bias)
    res = run_bass_kernel_spmd(nc, in_maps, core_ids=list(range(8)))
    out = np.stack([res.results[i]["y"].reshape(32, 32, 32) for i in range(8)])
    return out.astype(np.float32)


if __name__ == "__main__":
    rng = np.random.default_rng(0)
    x = rng.standard_normal((8, 32, 32, 32)).astype(np.float32)
    w = (rng.standard_normal((32, 32, 3, 3)) * 0.1).astype(np.float32)
    b = (rng.standard_normal(32) * 0.01).astype(np.float32)
    i = np.arange(256, dtype=np.float32) - 128.0
    lut = i[:, None] * i[None, :]
    out = kernel(x, w, b, lut)
    print(out.shape, out.dtype, float(np.abs(out).max()))
